# revision 36
# baseline (speedup 1.0000x reference)
"""Multi-head attention (B=4, S=2048, D=1024, H=16) on 8 trn2 NeuronCores.

Sharding: core c = (batch b, head-group g) with b in 0..3, g in 0..1.
Each core computes 8 heads of one batch; the two cores of a batch produce
partial output projections that the host sums.

All device tensors are kept in "transposed" layouts (feature dim on SBUF
partitions) so no on-device transposes are needed:
  Q^T/K^T [d, s], V [s, d], scores^T [k, q], o^T [d, q], y^T [out, q].

The two heads of a pair share each PE pass: QK^T is row-tiled over the two
K=64 halves of the partition dim (concurrent matmuls), and P·V uses two
128-column stationaries built from an overlapped V layout
  [V_A | 1 | 0...0 | 1 | V_B]   (193 columns per (kt, pair))
so both P·V matmuls keep Fast Weight Load (128-column weights) and carry
the softmax-denominator ones-column in fp32 PSUM for free:
  bank A out: o_A on partitions 0:64,  denom_A on partition 64
  bank B out: denom_B on partition 63, o_B on partitions 64:128.
Exp runs mostly on the scalar engine (ACT); a few tiles per iteration can
be offloaded to the vector engine via a Schraudolph-style bf16 bit-trick
exp to balance engine load.
"""
import math

import numpy as np
import ml_dtypes

import concourse.bass as bass
import concourse.mybir as mybir
import concourse.tile as tile
from concourse import bacc
from concourse.bass_utils import run_bass_kernel_spmd

B, S, D, H = 4, 2048, 1024, 16
DK = D // H              # 64
NCORES = 8
HG = 2                   # head groups (tensor-parallel axis)
HPG = H // HG            # 8 heads per core
HD = HPG * DK            # 512 head-dim features per core
PAIRS = HPG // 2         # 4 head pairs (2 heads packed per PE pass)
P = 128
VW = 194                 # packed V: [V_A |1| zeros |1| V_B] (B at 4B-aligned col 66)
QC = 512                 # q-chunk (matmul moving free dim)
NQC = S // QC            # 4
NKT = S // P             # 16 k-tiles
FK = D // P              # 8 feature c-tiles for projections
TC = 512                 # token chunk for QKV phase
NTC = S // TC            # 4

F32 = mybir.dt.float32
BF16 = mybir.dt.bfloat16
I16 = mybir.dt.int16

# Schraudolph bf16 exp: bits(exp(x)) ~= int16(x * 128*log2(e) + (127*128 - C))
SCH_A = 128.0 * 1.4426950408889634
SCH_B = 127.0 * 128.0 - 4.74

LAST_EXEC_NS = None


def _build(apply_mask: bool, qkv_bias: bool, dve_kt=(5, 9, 12, 15),
           dbg=False,
           dbg_it=(0, 0)):
    nc = bacc.Bacc("TRN2", debug=False, num_devices=NCORES)
    xT = nc.declare_dram_parameter("xT", [P, NTC * FK * TC], BF16,
                                   isOutput=False)
    wqkv = nc.declare_dram_parameter("wqkv", [P, 3 * FK * HD], BF16,
                                    isOutput=False)
    wo = nc.declare_dram_parameter("wo", [HD, D], BF16, isOutput=False)
    yT = nc.declare_dram_parameter("yT", [D, S], F32, isOutput=True)
    if dbg:
        dbg_den = nc.declare_dram_parameter("dbg_den", [2, QC], F32,
                                            isOutput=True)
        dbg_r = nc.declare_dram_parameter("dbg_r", [2, QC], F32,
                                          isOutput=True)
        dbg_o = nc.declare_dram_parameter("dbg_o", [P, QC], F32,
                                          isOutput=True)
    if apply_mask:
        maskT = nc.declare_dram_parameter("maskT", [S, S], F32, isOutput=False)
        dve_kt = ()          # keep the masked path simple: all exp on ACT
    if qkv_bias:
        qkb = nc.declare_dram_parameter("qkb", [2, HD], F32, isOutput=False)
        vb = nc.declare_dram_parameter("vb", [HD], F32, isOutput=False)
    dve_kt = set(dve_kt)

    xT_r = xT.rearrange("p (q fo t) -> p q fo t", q=NTC, fo=FK)
    wqkv_r = wqkv.rearrange("p (th fo j) -> p th fo j", th=3, fo=FK)
    wo_r = wo.rearrange("(co p) n -> p co n", p=P)       # [128, 4, 1024]
    yT_r = yT.rearrange("(oo p) s -> p oo s", p=P)       # [128, 8, 2048]

    phat_bufs = 1 if apply_mask else 2

    with tile.TileContext(nc) as tc:
        with tc.tile_pool(name="persist", bufs=1) as persist, \
             tc.tile_pool(name="work", bufs=1) as work, \
             tc.tile_pool(name="small", bufs=1) as small, \
             tc.tile_pool(name="phat", bufs=phat_bufs) as phatp, \
             tc.tile_pool(name="opool", bufs=2) as opool, \
             tc.tile_pool(name="ps", bufs=2, space="PSUM") as ps:

            QT = persist.tile([P, PAIRS, S], BF16)        # 16KB/part
            KTt = persist.tile([P, PAIRS, S], BF16)       # 16KB/part
            # packed V for the ones-column PV stationaries (24.1KB/part)
            Vp = persist.tile([P, NKT, PAIRS, VW], BF16)
            wo_t = persist.tile([P, HD // P, D], BF16)    # 8KB/part

            # ones + shared-zeros columns of the packed V
            nc.vector.memset(Vp[:, :, :, DK:2 * DK + 2], 0.0)
            nc.vector.memset(Vp[:, :, :, DK], 1.0)
            nc.vector.memset(Vp[:, :, :, 2 * DK + 1], 1.0)

            if qkv_bias:
                qkb_t = persist.tile([P, 2, PAIRS], F32)
                nc.sync.dma_start(
                    qkb_t, qkb.rearrange("t (pr p) -> p t pr", p=P))
                vb_bc = persist.tile([P, HD], F32)
                nc.sync.dma_start(vb_bc, vb[None, :].partition_broadcast(P))

            # x and weights stay resident as per-ko tiles (fine-grained DMA
            # deps, no pool-close barriers anywhere)
            x_big = persist.tile([P, NTC, FK, TC], BF16, name="x_big")
            w_big = persist.tile([P, 3, FK, HD], BF16, name="w_big")
            # one fully-contiguous DMA per x quarter / w third
            nc.sync.dma_start(w_big[:, 1, :, 0:P], wqkv_r[:, 1, :, 0:P])
            nc.sync.dma_start(x_big[:, 0], xT_r[:, 0])
            nc.sync.dma_start(w_big[:, 1, :, P:HD], wqkv_r[:, 1, :, P:HD])
            nc.sync.dma_start(x_big[:, 1], xT_r[:, 1])
            nc.sync.dma_start(w_big[:, 0], wqkv_r[:, 0])
            nc.sync.dma_start(w_big[:, 2], wqkv_r[:, 2])
            nc.sync.dma_start(x_big[:, 2], xT_r[:, 2])
            nc.sync.dma_start(x_big[:, 3], xT_r[:, 3])
            nc.sync.dma_start(wo_t, wo_r)

            def emit_qk_group(which, pair, tcix):
                tsl = slice(tcix * TC, (tcix + 1) * TC)
                psqk = ps.tile([P, 2 * QC], F32, tag="o", name="psqk")[:, :TC]
                msl = slice(pair * P, (pair + 1) * P)
                for ko in range(FK):
                    nc.tensor.matmul(
                        psqk, w_big[:, which, ko, msl],
                        x_big[:, tcix, ko], start=(ko == 0),
                        stop=(ko == FK - 1))
                dst = (QT if which == 0 else KTt)[:, pair, tsl]
                if qkv_bias:
                    nc.vector.tensor_scalar_add(
                        dst, psqk, qkb_t[:, which, pair, None])
                else:
                    nc.vector.tensor_copy(dst, psqk)

            def emit_v_group(kt):
                psv = ps.tile([P, 2 * QC], F32, tag="o", name="psv")[:, :HD]
                ql, off = kt // 4, (kt % 4) * P
                for ko in range(FK):
                    nc.tensor.matmul(
                        psv, x_big[:, ql, ko, off:off + P],
                        w_big[:, 2, ko], start=(ko == 0),
                        stop=(ko == FK - 1))
                vsrc = psv.rearrange("p (pr t c) -> p pr t c", pr=PAIRS, t=2)
                if qkv_bias:
                    vbr = vb_bc.rearrange(
                        "p (pr t c) -> p pr t c", pr=PAIRS, t=2)
                    nc.vector.tensor_add(
                        Vp[:, kt, :, 0:DK], vsrc[:, :, 0], vbr[:, :, 0])
                    nc.vector.tensor_add(
                        Vp[:, kt, :, 2 * DK + 2:VW], vsrc[:, :, 1],
                        vbr[:, :, 1])
                else:
                    nc.vector.tensor_copy(
                        Vp[:, kt, :, 0:DK], vsrc[:, :, 0])
                    nc.vector.tensor_copy(
                        Vp[:, kt, :, 2 * DK + 2:VW], vsrc[:, :, 1])

            o_tiles = {}

            def emit_proj_group(qc, oc):
                qsl = slice(qc * QC, (qc + 1) * QC)
                psy = ps.tile([P, 2 * QC], F32, tag="o", name="psy")[:, :QC]
                for c in range(HD // P):
                    nc.tensor.matmul(
                        psy, wo_t[:, c, oc * P:(oc + 1) * P],
                        o_tiles[qc][:, c, :],
                        start=(c == 0), stop=(c == HD // P - 1))
                yst = work.tile([P, QC], F32, tag=f"y{oc % 2}")
                nc.vector.tensor_copy(yst, psy)
                nc.sync.dma_start(yT_r[:, oc, qsl], yst)

            def emit_tail(st, pod):
                # denominators: bank A partition 64, bank B partition 63
                rr = small.tile([DK + 1, 2 * QC], F32, tag="r")
                # rr row 0 is recip garbage; reuse it as the partition-0
                # staging slot for the two denominator reciprocals
                nc.vector.reciprocal_approx_fast(
                    rr[0:DK + 1, 0:QC], pod[0:DK + 1, 0:QC])
                nc.sync.dma_start(rr[0:1, 0:QC], rr[DK:DK + 1, 0:QC])
                nc.vector.reciprocal_approx_fast(
                    rr[0:DK, QC:2 * QC], pod[0:DK, QC:2 * QC])
                nc.sync.dma_start(rr[0:1, QC:2 * QC],
                                  rr[DK - 1:DK, QC:2 * QC])
                rbcA = small.tile([DK, QC], F32, tag="rbcA")
                rbcB = small.tile([P, QC], F32, tag="rbcB")
                nc.gpsimd.partition_broadcast(rbcA, rr[0:1, 0:QC])
                nc.gpsimd.partition_broadcast(rbcB, rr[0:1, QC:2 * QC])
                nc.vector.tensor_mul(
                    o_tiles[st["qc"]][0:DK, st["pair"], :],
                    pod[0:DK, 0:QC], rbcA)
                nc.vector.tensor_mul(
                    o_tiles[st["qc"]][DK:P, st["pair"], :],
                    pod[DK:P, QC:2 * QC], rbcB[DK:P])
                if dbg and (st["qc"], st["pair"]) == dbg_it:
                    dent = work.tile([P, QC], F32, tag="y0")
                    nc.vector.tensor_copy(dent[0:DK + 1, :],
                                          pod[0:DK + 1, 0:QC])
                    nc.sync.dma_start(dbg_den[0:1, :], dent[DK:DK + 1])
                    nc.vector.tensor_copy(dent[0:DK, :],
                                          pod[0:DK, QC:2 * QC])
                    nc.sync.dma_start(dbg_den[1:2, :], dent[DK - 1:DK])
                    nc.sync.dma_start(dbg_r[0:1, :], rr[0:1, 0:QC])
                    nc.sync.dma_start(dbg_r[1:2, :], rr[0:1, QC:2 * QC])
                    ot = work.tile([P, QC], F32, tag="y0")
                    nc.vector.tensor_copy(
                        ot, o_tiles[st["qc"]][:, st["pair"], :])
                    nc.sync.dma_start(dbg_o[:, :], ot)

            # ---- prologue: K^T (all chunks) + Q^T (chunk 0) -------------
            for tcix in range(NTC):
                for pair in range(PAIRS):
                    emit_qk_group(1, pair, tcix)
                if tcix == 0:
                    for pair in range(PAIRS):
                        emit_qk_group(0, pair, 0)

            # V and remaining Q drain into the attention kt-stream
            pending = []   # (ready_iter, late_only, fn)
            for kt in range(NKT):
                pending.append((0, False, (lambda kt=kt: emit_v_group(kt))))
            for tcix in range(1, NTC):
                for pair in range(PAIRS):
                    pending.append(
                        (0, False, (lambda pair=pair, tcix=tcix:
                                    emit_qk_group(0, pair, tcix))))
            it_idx = 0

            def drain(limit, allow_late=True):
                n = 0
                while (pending and n < limit and pending[0][0] <= it_idx
                       and (allow_late or not pending[0][1])):
                    _, _, fn = pending.pop(0)
                    fn()
                    n += 1

            def emit_pv(st, pod, kt, first, last):
                pv = st["phat"][kt]
                vk = Vp[:, kt, st["pair"]]
                nc.tensor.matmul(
                    pod[:, 0:QC], vk[:, 0:P], pv[:, 0:QC],
                    start=first, stop=last)
                nc.tensor.matmul(
                    pod[:, QC:2 * QC], vk[:, DK + 2:VW], pv[:, QC:2 * QC],
                    start=first, stop=last)

            # ---- main attention pipeline --------------------------------
            prev = None
            for qc in range(NQC):
                qsl = slice(qc * QC, (qc + 1) * QC)
                o_tiles[qc] = opool.tile(
                    [P, HD // P, QC], BF16, tag="o_sb", name="o_sb")
                if apply_mask:
                    mt = opool.tile([P, NKT, QC], F32, tag="mask")
                    nc.sync.dma_start(
                        mt,
                        maskT.rearrange("(ko p) q -> p ko q", p=P)[:, :, qsl])
                for pair in range(PAIRS):
                    phat = [phatp.tile([P, 2 * QC], BF16, tag=f"ph{k}",
                                       name=f"ph{k}") for k in range(NKT)]
                    pod = (ps.tile([P, 2 * QC], F32, tag="o", name="pod")
                           if prev is not None else None)
                    for kt2 in range(0, NKT, 2):
                        for kt in (kt2, kt2 + 1):
                            ksl = slice(kt * P, (kt + 1) * P)
                            pss = ps.tile([P, 2 * QC], F32, tag="scores",
                                          name=f"pss{kt & 1}")
                            nc.tensor.matmul(
                                pss[:, 0:QC], KTt[0:DK, pair, ksl],
                                QT[0:DK, pair, qsl], start=True, stop=True)
                            nc.tensor.matmul(
                                pss[:, QC:2 * QC], KTt[DK:P, pair, ksl],
                                QT[DK:P, pair, qsl], start=True, stop=True)
                            if apply_mask:
                                nc.vector.tensor_add(
                                    pss[:, 0:QC], pss[:, 0:QC], mt[:, kt])
                                nc.vector.tensor_add(
                                    pss[:, QC:2 * QC], pss[:, QC:2 * QC],
                                    mt[:, kt])
                            if kt in dve_kt:
                                nc.vector.tensor_scalar(
                                    phat[kt].bitcast(I16), pss, SCH_A, SCH_B,
                                    mybir.AluOpType.mult,
                                    mybir.AluOpType.add)
                            else:
                                nc.scalar.activation(
                                    phat[kt], pss,
                                    mybir.ActivationFunctionType.Exp)
                        if it_idx <= 1:
                            drain(1, allow_late=(prev is None or kt2 >= 5))
                        elif (kt2 // 2) % 2 == 0:
                            drain(1, allow_late=(kt2 >= 5))
                        if prev is not None:
                            emit_pv(prev, pod, kt2, kt2 == 0, False)
                            emit_pv(prev, pod, kt2 + 1, False,
                                    kt2 + 1 == NKT - 1)
                    if prev is not None:
                        emit_tail(prev, pod)
                        if prev["pair"] == PAIRS - 1:
                            pending.extend(
                                (it_idx + 1, True,
                                 (lambda pqc=prev["qc"], oc=oc:
                                  emit_proj_group(pqc, oc)))
                                for oc in range(D // P))
                    prev = {"qc": qc, "pair": pair, "phat": phat}
                    it_idx += 1

            # epilogue
            pod = ps.tile([P, 2 * QC], F32, tag="o", name="pod")
            it_idx += 100
            for kt in range(NKT):
                emit_pv(prev, pod, kt, kt == 0, kt == NKT - 1)
            emit_tail(prev, pod)
            while pending:
                _, _, fn = pending.pop(0)
                fn()
            for oc in range(D // P):
                emit_proj_group(prev["qc"], oc)

    nc.finalize()
    return nc


# --------------------------------------------------------------------------
# NTFF profiling shim (only used when kernel(..., _trace=True); provides
# antenv.axon_hooks so run_bass_kernel_spmd can capture profiles under axon).
def _install_ntff_shim():
    import contextlib, ctypes, sys, types
    try:
        import antenv.axon_hooks  # noqa: F401
        return
    except ImportError:
        pass
    so = "/opt/axon/libaxon_pjrt.so"
    try:
        lib = ctypes.CDLL(so)
    except OSError:
        return
    if not hasattr(lib, "axon_start_nrt_profile"):
        return
    lib.axon_start_nrt_profile.argtypes = [
        ctypes.POINTER(ctypes.c_int64), ctypes.c_size_t]
    lib.axon_start_nrt_profile.restype = ctypes.c_int64
    lib.axon_stop_nrt_profile.argtypes = [ctypes.c_char_p]
    lib.axon_stop_nrt_profile.restype = ctypes.c_int64

    @contextlib.contextmanager
    def _hook(output_dir, device_ids):
        import jax
        jax.devices()
        if device_ids:
            ids = (ctypes.c_int64 * len(device_ids))(*device_ids)
            rc = lib.axon_start_nrt_profile(ids, len(device_ids))
        else:
            rc = lib.axon_start_nrt_profile(None, 0)
        if rc != 0:
            raise RuntimeError(f"axon_start_nrt_profile rc={rc}")
        try:
            yield
        finally:
            n = lib.axon_stop_nrt_profile(str(output_dir).encode())
            print(f"ntff: {n} profile file(s) in {output_dir}", file=sys.stderr)

    import antenv
    mod = types.ModuleType("antenv.axon_hooks")
    mod.get_axon_ntff_profile_hook = lambda: _hook
    mod.set_axon_ntff_profile_hook = lambda h: None
    sys.modules["antenv.axon_hooks"] = mod
    antenv.axon_hooks = mod


def kernel(x, mask, Wq, bq, Wk, bk, Wv, bv, Wo, bo, _trace=False):
    global LAST_EXEC_NS
    x = np.ascontiguousarray(np.asarray(x, dtype=np.float32))
    mask = np.asarray(mask)
    Wq = np.asarray(Wq, dtype=np.float32)
    Wk = np.asarray(Wk, dtype=np.float32)
    Wv = np.asarray(Wv, dtype=np.float32)
    Wo = np.asarray(Wo, dtype=np.float32)
    bq = np.asarray(bq, dtype=np.float32)
    bk = np.asarray(bk, dtype=np.float32)
    bv = np.asarray(bv, dtype=np.float32)
    bo = np.asarray(bo, dtype=np.float32)

    scale = np.float32(1.0 / math.sqrt(DK))
    apply_mask = not bool((mask != 0).all())
    qkv_bias = bool(bq.any() or bk.any() or bv.any())

    import os
    dve_kt = tuple(
        int(t) for t in os.environ.get("DVE_KT", "5,9,12,15").split(",") if t)
    dbg = bool(os.environ.get("KDBG"))
    dbg_it = tuple(int(t) for t in os.environ.get("DBGIT", "0,0").split(","))
    nc = _build(apply_mask, qkv_bias, dve_kt=dve_kt, dbg=dbg, dbg_it=dbg_it)

    if apply_mask:
        mbias = np.where(mask == 0, np.float32(-1e9), np.float32(0.0))
        # maskT[b][k, q] = mbias[b][q, k]
        maskT = np.ascontiguousarray(np.transpose(mbias, (0, 2, 1)))

    in_maps = []
    for b in range(B):
        # pack x as [p, quarter, ko, t] so each quarter is one contiguous DMA
        xT_np = np.ascontiguousarray(
            x[b].reshape(NTC, TC, FK, P).transpose(3, 0, 2, 1)
            .reshape(P, -1)).astype(ml_dtypes.bfloat16)
        for g in range(HG):
            rows = slice(g * HD, (g + 1) * HD)
            wc = np.concatenate(
                [Wq[rows].T * scale, Wk[rows].T, Wv[rows].T],
                axis=1)   # [D, 3*HD], thirds Q|K|V
            wqkv_np = np.ascontiguousarray(
                wc.reshape(FK, P, 3, HD).transpose(1, 2, 0, 3)
                .reshape(P, -1)).astype(ml_dtypes.bfloat16)
            wo_np = np.ascontiguousarray(
                Wo[:, rows].T).astype(ml_dtypes.bfloat16)
            m = {"xT": xT_np, "wqkv": wqkv_np, "wo": wo_np}
            if apply_mask:
                m["maskT"] = maskT[b]
            if qkv_bias:
                m["qkb"] = np.ascontiguousarray(
                    np.stack([bq[rows] * scale, bk[rows]]))
                m["vb"] = np.ascontiguousarray(bv[rows])
            in_maps.append(m)

    if _trace:
        _install_ntff_shim()
    r = run_bass_kernel_spmd(nc, in_maps, list(range(NCORES)), trace=_trace)
    LAST_EXEC_NS = r.exec_time_ns
    if dbg:
        global DBG_OUT
        DBG_OUT = r.results

    y = np.empty((B, S, D), dtype=np.float32)
    for b in range(B):
        yT = r.results[2 * b]["yT"] + r.results[2 * b + 1]["yT"]
        y[b] = yT.T + bo[None, :]
    return y


# revision 37
# speedup vs baseline: 1.0056x; 1.0056x over previous
"""Multi-head attention (B=4, S=2048, D=1024, H=16) on 8 trn2 NeuronCores.

Sharding: core c = (batch b, head-group g) with b in 0..3, g in 0..1.
Each core computes 8 heads of one batch; the two cores of a batch produce
partial output projections that the host sums.

All device tensors are kept in "transposed" layouts (feature dim on SBUF
partitions) so no on-device transposes are needed:
  Q^T/K^T [d, s], V [s, d], scores^T [k, q], o^T [d, q], y^T [out, q].

The two heads of a pair share each PE pass: QK^T is row-tiled over the two
K=64 halves of the partition dim (concurrent matmuls), and P·V uses two
128-column stationaries built from an overlapped V layout
  [V_A | 1 | 0...0 | 1 | V_B]   (193 columns per (kt, pair))
so both P·V matmuls keep Fast Weight Load (128-column weights) and carry
the softmax-denominator ones-column in fp32 PSUM for free:
  bank A out: o_A on partitions 0:64,  denom_A on partition 64
  bank B out: denom_B on partition 63, o_B on partitions 64:128.
Exp runs mostly on the scalar engine (ACT); a few tiles per iteration can
be offloaded to the vector engine via a Schraudolph-style bf16 bit-trick
exp to balance engine load.
"""
import math

import numpy as np
import ml_dtypes

import concourse.bass as bass
import concourse.mybir as mybir
import concourse.tile as tile
from concourse import bacc
from concourse.bass_utils import run_bass_kernel_spmd

B, S, D, H = 4, 2048, 1024, 16
DK = D // H              # 64
NCORES = 8
HG = 2                   # head groups (tensor-parallel axis)
HPG = H // HG            # 8 heads per core
HD = HPG * DK            # 512 head-dim features per core
PAIRS = HPG // 2         # 4 head pairs (2 heads packed per PE pass)
P = 128
VW = 194                 # packed V: [V_A |1| zeros |1| V_B] (B at 4B-aligned col 66)
QC = 512                 # q-chunk (matmul moving free dim)
NQC = S // QC            # 4
NKT = S // P             # 16 k-tiles
FK = D // P              # 8 feature c-tiles for projections
TC = 512                 # token chunk for QKV phase
NTC = S // TC            # 4

F32 = mybir.dt.float32
BF16 = mybir.dt.bfloat16
I16 = mybir.dt.int16

# Schraudolph bf16 exp: bits(exp(x)) ~= int16(x * 128*log2(e) + (127*128 - C))
SCH_A = 128.0 * 1.4426950408889634
SCH_B = 127.0 * 128.0 - 4.74

LAST_EXEC_NS = None


def _build(apply_mask: bool, qkv_bias: bool, dve_kt=(5, 9, 12, 15),
           dbg=False,
           dbg_it=(0, 0)):
    nc = bacc.Bacc("TRN2", debug=False, num_devices=NCORES)
    xT = nc.declare_dram_parameter("xT", [P, NTC * FK * TC], BF16,
                                   isOutput=False)
    wqkv = nc.declare_dram_parameter("wqkv", [P, 3 * FK * HD], BF16,
                                    isOutput=False)
    wo = nc.declare_dram_parameter("wo", [HD, D], BF16, isOutput=False)
    yT = nc.declare_dram_parameter("yT", [D, S], BF16, isOutput=True)
    if dbg:
        dbg_den = nc.declare_dram_parameter("dbg_den", [2, QC], F32,
                                            isOutput=True)
        dbg_r = nc.declare_dram_parameter("dbg_r", [2, QC], F32,
                                          isOutput=True)
        dbg_o = nc.declare_dram_parameter("dbg_o", [P, QC], F32,
                                          isOutput=True)
    if apply_mask:
        maskT = nc.declare_dram_parameter("maskT", [S, S], F32, isOutput=False)
        dve_kt = ()          # keep the masked path simple: all exp on ACT
    if qkv_bias:
        qkb = nc.declare_dram_parameter("qkb", [2, HD], F32, isOutput=False)
        vb = nc.declare_dram_parameter("vb", [HD], F32, isOutput=False)
    dve_kt = set(dve_kt)

    xT_r = xT.rearrange("p (q fo t) -> p q fo t", q=NTC, fo=FK)
    wqkv_r = wqkv.rearrange("p (th fo j) -> p th fo j", th=3, fo=FK)
    wo_r = wo.rearrange("(co p) n -> p co n", p=P)       # [128, 4, 1024]
    yT_r = yT.rearrange("(oo p) s -> p oo s", p=P)       # [128, 8, 2048]

    phat_bufs = 1 if apply_mask else 2

    with tile.TileContext(nc) as tc:
        with tc.tile_pool(name="persist", bufs=1) as persist, \
             tc.tile_pool(name="work", bufs=1) as work, \
             tc.tile_pool(name="small", bufs=1) as small, \
             tc.tile_pool(name="phat", bufs=phat_bufs) as phatp, \
             tc.tile_pool(name="opool", bufs=2) as opool, \
             tc.tile_pool(name="ps", bufs=2, space="PSUM") as ps:

            QT = persist.tile([P, PAIRS, S], BF16)        # 16KB/part
            KTt = persist.tile([P, PAIRS, S], BF16)       # 16KB/part
            # packed V for the ones-column PV stationaries (24.1KB/part)
            Vp = persist.tile([P, NKT, PAIRS, VW], BF16)
            wo_t = persist.tile([P, HD // P, D], BF16)    # 8KB/part

            # ones + shared-zeros columns of the packed V
            nc.vector.memset(Vp[:, :, :, DK:2 * DK + 2], 0.0)
            nc.vector.memset(Vp[:, :, :, DK], 1.0)
            nc.vector.memset(Vp[:, :, :, 2 * DK + 1], 1.0)

            if qkv_bias:
                qkb_t = persist.tile([P, 2, PAIRS], F32)
                nc.sync.dma_start(
                    qkb_t, qkb.rearrange("t (pr p) -> p t pr", p=P))
                vb_bc = persist.tile([P, HD], F32)
                nc.sync.dma_start(vb_bc, vb[None, :].partition_broadcast(P))

            # x and weights stay resident as per-ko tiles (fine-grained DMA
            # deps, no pool-close barriers anywhere)
            x_big = persist.tile([P, NTC, FK, TC], BF16, name="x_big")
            w_big = persist.tile([P, 3, FK, HD], BF16, name="w_big")
            # one fully-contiguous DMA per x quarter / w third
            nc.sync.dma_start(w_big[:, 1, :, 0:P], wqkv_r[:, 1, :, 0:P])
            nc.sync.dma_start(x_big[:, 0], xT_r[:, 0])
            nc.sync.dma_start(w_big[:, 1, :, P:HD], wqkv_r[:, 1, :, P:HD])
            nc.sync.dma_start(x_big[:, 1], xT_r[:, 1])
            nc.sync.dma_start(w_big[:, 0], wqkv_r[:, 0])
            nc.sync.dma_start(w_big[:, 2], wqkv_r[:, 2])
            nc.sync.dma_start(x_big[:, 2], xT_r[:, 2])
            nc.sync.dma_start(x_big[:, 3], xT_r[:, 3])
            nc.sync.dma_start(wo_t, wo_r)

            def emit_qk_group(which, pair, tcix):
                tsl = slice(tcix * TC, (tcix + 1) * TC)
                psqk = ps.tile([P, 2 * QC], F32, tag="o", name="psqk")[:, :TC]
                msl = slice(pair * P, (pair + 1) * P)
                for ko in range(FK):
                    nc.tensor.matmul(
                        psqk, w_big[:, which, ko, msl],
                        x_big[:, tcix, ko], start=(ko == 0),
                        stop=(ko == FK - 1))
                dst = (QT if which == 0 else KTt)[:, pair, tsl]
                if qkv_bias:
                    nc.vector.tensor_scalar_add(
                        dst, psqk, qkb_t[:, which, pair, None])
                else:
                    nc.vector.tensor_copy(dst, psqk)

            def emit_v_group(kt):
                psv = ps.tile([P, 2 * QC], F32, tag="o", name="psv")[:, :HD]
                ql, off = kt // 4, (kt % 4) * P
                for ko in range(FK):
                    nc.tensor.matmul(
                        psv, x_big[:, ql, ko, off:off + P],
                        w_big[:, 2, ko], start=(ko == 0),
                        stop=(ko == FK - 1))
                vsrc = psv.rearrange("p (pr t c) -> p pr t c", pr=PAIRS, t=2)
                if qkv_bias:
                    vbr = vb_bc.rearrange(
                        "p (pr t c) -> p pr t c", pr=PAIRS, t=2)
                    nc.vector.tensor_add(
                        Vp[:, kt, :, 0:DK], vsrc[:, :, 0], vbr[:, :, 0])
                    nc.vector.tensor_add(
                        Vp[:, kt, :, 2 * DK + 2:VW], vsrc[:, :, 1],
                        vbr[:, :, 1])
                else:
                    nc.vector.tensor_copy(
                        Vp[:, kt, :, 0:DK], vsrc[:, :, 0])
                    nc.vector.tensor_copy(
                        Vp[:, kt, :, 2 * DK + 2:VW], vsrc[:, :, 1])

            o_tiles = {}

            def emit_proj_group(qc, oc):
                qsl = slice(qc * QC, (qc + 1) * QC)
                psy = ps.tile([P, 2 * QC], F32, tag="o", name="psy")[:, :QC]
                for c in range(HD // P):
                    nc.tensor.matmul(
                        psy, wo_t[:, c, oc * P:(oc + 1) * P],
                        o_tiles[qc][:, c, :],
                        start=(c == 0), stop=(c == HD // P - 1))
                yst = work.tile([P, QC], BF16, tag=f"y{oc % 2}")
                nc.vector.tensor_copy(yst, psy)
                nc.sync.dma_start(yT_r[:, oc, qsl], yst)

            def emit_tail(st, pod):
                # denominators: bank A partition 64, bank B partition 63
                rr = small.tile([DK + 1, 2 * QC], F32, tag="r")
                # rr row 0 is recip garbage; reuse it as the partition-0
                # staging slot for the two denominator reciprocals
                nc.vector.reciprocal_approx_fast(
                    rr[0:DK + 1, 0:QC], pod[0:DK + 1, 0:QC])
                nc.sync.dma_start(rr[0:1, 0:QC], rr[DK:DK + 1, 0:QC])
                nc.vector.reciprocal_approx_fast(
                    rr[0:DK, QC:2 * QC], pod[0:DK, QC:2 * QC])
                nc.sync.dma_start(rr[0:1, QC:2 * QC],
                                  rr[DK - 1:DK, QC:2 * QC])
                rbcA = small.tile([DK, QC], F32, tag="rbcA")
                rbcB = small.tile([P, QC], F32, tag="rbcB")
                nc.gpsimd.partition_broadcast(rbcA, rr[0:1, 0:QC])
                nc.gpsimd.partition_broadcast(rbcB, rr[0:1, QC:2 * QC])
                nc.vector.tensor_mul(
                    o_tiles[st["qc"]][0:DK, st["pair"], :],
                    pod[0:DK, 0:QC], rbcA)
                nc.vector.tensor_mul(
                    o_tiles[st["qc"]][DK:P, st["pair"], :],
                    pod[DK:P, QC:2 * QC], rbcB[DK:P])
                if dbg and (st["qc"], st["pair"]) == dbg_it:
                    dent = work.tile([P, QC], F32, tag="y0")
                    nc.vector.tensor_copy(dent[0:DK + 1, :],
                                          pod[0:DK + 1, 0:QC])
                    nc.sync.dma_start(dbg_den[0:1, :], dent[DK:DK + 1])
                    nc.vector.tensor_copy(dent[0:DK, :],
                                          pod[0:DK, QC:2 * QC])
                    nc.sync.dma_start(dbg_den[1:2, :], dent[DK - 1:DK])
                    nc.sync.dma_start(dbg_r[0:1, :], rr[0:1, 0:QC])
                    nc.sync.dma_start(dbg_r[1:2, :], rr[0:1, QC:2 * QC])
                    ot = work.tile([P, QC], F32, tag="y0")
                    nc.vector.tensor_copy(
                        ot, o_tiles[st["qc"]][:, st["pair"], :])
                    nc.sync.dma_start(dbg_o[:, :], ot)

            # ---- prologue: K^T (all chunks) + Q^T (chunk 0) -------------
            for tcix in range(NTC):
                for pair in range(PAIRS):
                    emit_qk_group(1, pair, tcix)
                if tcix == 0:
                    for pair in range(PAIRS):
                        emit_qk_group(0, pair, 0)

            # V and remaining Q drain into the attention kt-stream
            pending = []   # (ready_iter, late_only, fn)
            for kt in range(NKT):
                pending.append((0, False, (lambda kt=kt: emit_v_group(kt))))
            for tcix in range(1, NTC):
                for pair in range(PAIRS):
                    pending.append(
                        (0, False, (lambda pair=pair, tcix=tcix:
                                    emit_qk_group(0, pair, tcix))))
            it_idx = 0

            def drain(limit, allow_late=True):
                n = 0
                while (pending and n < limit and pending[0][0] <= it_idx
                       and (allow_late or not pending[0][1])):
                    _, _, fn = pending.pop(0)
                    fn()
                    n += 1

            def emit_pv(st, pod, kt, first, last):
                pv = st["phat"][kt]
                vk = Vp[:, kt, st["pair"]]
                nc.tensor.matmul(
                    pod[:, 0:QC], vk[:, 0:P], pv[:, 0:QC],
                    start=first, stop=last)
                nc.tensor.matmul(
                    pod[:, QC:2 * QC], vk[:, DK + 2:VW], pv[:, QC:2 * QC],
                    start=first, stop=last)

            # ---- main attention pipeline --------------------------------
            prev = None
            for qc in range(NQC):
                qsl = slice(qc * QC, (qc + 1) * QC)
                o_tiles[qc] = opool.tile(
                    [P, HD // P, QC], BF16, tag="o_sb", name="o_sb")
                if apply_mask:
                    mt = opool.tile([P, NKT, QC], F32, tag="mask")
                    nc.sync.dma_start(
                        mt,
                        maskT.rearrange("(ko p) q -> p ko q", p=P)[:, :, qsl])
                for pair in range(PAIRS):
                    phat = [phatp.tile([P, 2 * QC], BF16, tag=f"ph{k}",
                                       name=f"ph{k}") for k in range(NKT)]
                    pod = (ps.tile([P, 2 * QC], F32, tag="o", name="pod")
                           if prev is not None else None)
                    for kt2 in range(0, NKT, 2):
                        for kt in (kt2, kt2 + 1):
                            ksl = slice(kt * P, (kt + 1) * P)
                            pss = ps.tile([P, 2 * QC], F32, tag="scores",
                                          name=f"pss{kt & 1}")
                            nc.tensor.matmul(
                                pss[:, 0:QC], KTt[0:DK, pair, ksl],
                                QT[0:DK, pair, qsl], start=True, stop=True)
                            nc.tensor.matmul(
                                pss[:, QC:2 * QC], KTt[DK:P, pair, ksl],
                                QT[DK:P, pair, qsl], start=True, stop=True)
                            if apply_mask:
                                nc.vector.tensor_add(
                                    pss[:, 0:QC], pss[:, 0:QC], mt[:, kt])
                                nc.vector.tensor_add(
                                    pss[:, QC:2 * QC], pss[:, QC:2 * QC],
                                    mt[:, kt])
                            if kt in dve_kt:
                                nc.vector.tensor_scalar(
                                    phat[kt].bitcast(I16), pss, SCH_A, SCH_B,
                                    mybir.AluOpType.mult,
                                    mybir.AluOpType.add)
                            else:
                                nc.scalar.activation(
                                    phat[kt], pss,
                                    mybir.ActivationFunctionType.Exp)
                        if it_idx <= 1:
                            drain(1, allow_late=(prev is None or kt2 >= 5))
                        elif (kt2 // 2) % 2 == 0:
                            drain(1, allow_late=(kt2 >= 5))
                        if prev is not None:
                            emit_pv(prev, pod, kt2, kt2 == 0, False)
                            emit_pv(prev, pod, kt2 + 1, False,
                                    kt2 + 1 == NKT - 1)
                    if prev is not None:
                        emit_tail(prev, pod)
                        if prev["pair"] == PAIRS - 1:
                            pending.extend(
                                (it_idx + 1, True,
                                 (lambda pqc=prev["qc"], oc=oc:
                                  emit_proj_group(pqc, oc)))
                                for oc in range(D // P))
                    prev = {"qc": qc, "pair": pair, "phat": phat}
                    it_idx += 1

            # epilogue
            pod = ps.tile([P, 2 * QC], F32, tag="o", name="pod")
            it_idx += 100
            for kt in range(NKT):
                emit_pv(prev, pod, kt, kt == 0, kt == NKT - 1)
            emit_tail(prev, pod)
            while pending:
                _, _, fn = pending.pop(0)
                fn()
            for oc in range(D // P):
                emit_proj_group(prev["qc"], oc)

    nc.finalize()
    return nc


# --------------------------------------------------------------------------
# NTFF profiling shim (only used when kernel(..., _trace=True); provides
# antenv.axon_hooks so run_bass_kernel_spmd can capture profiles under axon).
def _install_ntff_shim():
    import contextlib, ctypes, sys, types
    try:
        import antenv.axon_hooks  # noqa: F401
        return
    except ImportError:
        pass
    so = "/opt/axon/libaxon_pjrt.so"
    try:
        lib = ctypes.CDLL(so)
    except OSError:
        return
    if not hasattr(lib, "axon_start_nrt_profile"):
        return
    lib.axon_start_nrt_profile.argtypes = [
        ctypes.POINTER(ctypes.c_int64), ctypes.c_size_t]
    lib.axon_start_nrt_profile.restype = ctypes.c_int64
    lib.axon_stop_nrt_profile.argtypes = [ctypes.c_char_p]
    lib.axon_stop_nrt_profile.restype = ctypes.c_int64

    @contextlib.contextmanager
    def _hook(output_dir, device_ids):
        import jax
        jax.devices()
        if device_ids:
            ids = (ctypes.c_int64 * len(device_ids))(*device_ids)
            rc = lib.axon_start_nrt_profile(ids, len(device_ids))
        else:
            rc = lib.axon_start_nrt_profile(None, 0)
        if rc != 0:
            raise RuntimeError(f"axon_start_nrt_profile rc={rc}")
        try:
            yield
        finally:
            n = lib.axon_stop_nrt_profile(str(output_dir).encode())
            print(f"ntff: {n} profile file(s) in {output_dir}", file=sys.stderr)

    import antenv
    mod = types.ModuleType("antenv.axon_hooks")
    mod.get_axon_ntff_profile_hook = lambda: _hook
    mod.set_axon_ntff_profile_hook = lambda h: None
    sys.modules["antenv.axon_hooks"] = mod
    antenv.axon_hooks = mod


def kernel(x, mask, Wq, bq, Wk, bk, Wv, bv, Wo, bo, _trace=False):
    global LAST_EXEC_NS
    x = np.ascontiguousarray(np.asarray(x, dtype=np.float32))
    mask = np.asarray(mask)
    Wq = np.asarray(Wq, dtype=np.float32)
    Wk = np.asarray(Wk, dtype=np.float32)
    Wv = np.asarray(Wv, dtype=np.float32)
    Wo = np.asarray(Wo, dtype=np.float32)
    bq = np.asarray(bq, dtype=np.float32)
    bk = np.asarray(bk, dtype=np.float32)
    bv = np.asarray(bv, dtype=np.float32)
    bo = np.asarray(bo, dtype=np.float32)

    scale = np.float32(1.0 / math.sqrt(DK))
    apply_mask = not bool((mask != 0).all())
    qkv_bias = bool(bq.any() or bk.any() or bv.any())

    import os
    dve_kt = tuple(
        int(t) for t in os.environ.get("DVE_KT", "5,9,12,15").split(",") if t)
    dbg = bool(os.environ.get("KDBG"))
    dbg_it = tuple(int(t) for t in os.environ.get("DBGIT", "0,0").split(","))
    nc = _build(apply_mask, qkv_bias, dve_kt=dve_kt, dbg=dbg, dbg_it=dbg_it)

    if apply_mask:
        mbias = np.where(mask == 0, np.float32(-1e9), np.float32(0.0))
        # maskT[b][k, q] = mbias[b][q, k]
        maskT = np.ascontiguousarray(np.transpose(mbias, (0, 2, 1)))

    in_maps = []
    for b in range(B):
        # pack x as [p, quarter, ko, t] so each quarter is one contiguous DMA
        xT_np = np.ascontiguousarray(
            x[b].reshape(NTC, TC, FK, P).transpose(3, 0, 2, 1)
            .reshape(P, -1)).astype(ml_dtypes.bfloat16)
        for g in range(HG):
            rows = slice(g * HD, (g + 1) * HD)
            wc = np.concatenate(
                [Wq[rows].T * scale, Wk[rows].T, Wv[rows].T],
                axis=1)   # [D, 3*HD], thirds Q|K|V
            wqkv_np = np.ascontiguousarray(
                wc.reshape(FK, P, 3, HD).transpose(1, 2, 0, 3)
                .reshape(P, -1)).astype(ml_dtypes.bfloat16)
            wo_np = np.ascontiguousarray(
                Wo[:, rows].T).astype(ml_dtypes.bfloat16)
            m = {"xT": xT_np, "wqkv": wqkv_np, "wo": wo_np}
            if apply_mask:
                m["maskT"] = maskT[b]
            if qkv_bias:
                m["qkb"] = np.ascontiguousarray(
                    np.stack([bq[rows] * scale, bk[rows]]))
                m["vb"] = np.ascontiguousarray(bv[rows])
            in_maps.append(m)

    if _trace:
        _install_ntff_shim()
    r = run_bass_kernel_spmd(nc, in_maps, list(range(NCORES)), trace=_trace)
    LAST_EXEC_NS = r.exec_time_ns
    if dbg:
        global DBG_OUT
        DBG_OUT = r.results

    y = np.empty((B, S, D), dtype=np.float32)
    for b in range(B):
        yT = (r.results[2 * b]["yT"].astype(np.float32)
              + r.results[2 * b + 1]["yT"].astype(np.float32))
        y[b] = yT.T + bo[None, :]
    return y


# revision 38
# speedup vs baseline: 1.0062x; 1.0005x over previous
"""Multi-head attention (B=4, S=2048, D=1024, H=16) on 8 trn2 NeuronCores.

Sharding: core c = (batch b, head-group g) with b in 0..3, g in 0..1.
Each core computes 8 heads of one batch; the two cores of a batch produce
partial output projections that the host sums.

All device tensors are kept in "transposed" layouts (feature dim on SBUF
partitions) so no on-device transposes are needed:
  Q^T/K^T [d, s], V [s, d], scores^T [k, q], o^T [d, q], y^T [out, q].

The two heads of a pair share each PE pass: QK^T is row-tiled over the two
K=64 halves of the partition dim (concurrent matmuls), and P·V uses two
128-column stationaries built from an overlapped V layout
  [V_A | 1 | 0...0 | 1 | V_B]   (193 columns per (kt, pair))
so both P·V matmuls keep Fast Weight Load (128-column weights) and carry
the softmax-denominator ones-column in fp32 PSUM for free:
  bank A out: o_A on partitions 0:64,  denom_A on partition 64
  bank B out: denom_B on partition 63, o_B on partitions 64:128.
Exp runs mostly on the scalar engine (ACT); a few tiles per iteration can
be offloaded to the vector engine via a Schraudolph-style bf16 bit-trick
exp to balance engine load.
"""
import math

import numpy as np
import ml_dtypes

import concourse.bass as bass
import concourse.mybir as mybir
import concourse.tile as tile
from concourse import bacc
from concourse.bass_utils import run_bass_kernel_spmd

B, S, D, H = 4, 2048, 1024, 16
DK = D // H              # 64
NCORES = 8
HG = 2                   # head groups (tensor-parallel axis)
HPG = H // HG            # 8 heads per core
HD = HPG * DK            # 512 head-dim features per core
PAIRS = HPG // 2         # 4 head pairs (2 heads packed per PE pass)
P = 128
VW = 194                 # packed V: [V_A |1| zeros |1| V_B] (B at 4B-aligned col 66)
QC = 512                 # q-chunk (matmul moving free dim)
NQC = S // QC            # 4
NKT = S // P             # 16 k-tiles
FK = D // P              # 8 feature c-tiles for projections
TC = 512                 # token chunk for QKV phase
NTC = S // TC            # 4

F32 = mybir.dt.float32
BF16 = mybir.dt.bfloat16
I16 = mybir.dt.int16

# Schraudolph bf16 exp: bits(exp(x)) ~= int16(x * 128*log2(e) + (127*128 - C))
SCH_A = 128.0 * 1.4426950408889634
SCH_B = 127.0 * 128.0 - 4.74

LAST_EXEC_NS = None


def _build(apply_mask: bool, qkv_bias: bool, dve_kt=(5, 9, 12, 15),
           dbg=False,
           dbg_it=(0, 0)):
    nc = bacc.Bacc("TRN2", debug=False, num_devices=NCORES)
    xT = nc.declare_dram_parameter("xT", [P, NTC * FK * TC], BF16,
                                   isOutput=False)
    wqkv = nc.declare_dram_parameter("wqkv", [P, 3 * FK * HD], BF16,
                                    isOutput=False)
    wo = nc.declare_dram_parameter("wo", [HD, D], BF16, isOutput=False)
    yT = nc.declare_dram_parameter("yT", [D, S], BF16, isOutput=True)
    if dbg:
        dbg_den = nc.declare_dram_parameter("dbg_den", [2, QC], F32,
                                            isOutput=True)
        dbg_r = nc.declare_dram_parameter("dbg_r", [2, QC], F32,
                                          isOutput=True)
        dbg_o = nc.declare_dram_parameter("dbg_o", [P, QC], F32,
                                          isOutput=True)
    if apply_mask:
        maskT = nc.declare_dram_parameter("maskT", [S, S], F32, isOutput=False)
        dve_kt = ()          # keep the masked path simple: all exp on ACT
    if qkv_bias:
        qkb = nc.declare_dram_parameter("qkb", [2, HD], F32, isOutput=False)
        vb = nc.declare_dram_parameter("vb", [HD], F32, isOutput=False)
    dve_kt = set(dve_kt)

    xT_r = xT.rearrange("p (q fo t) -> p q fo t", q=NTC, fo=FK)
    wqkv_r = wqkv.rearrange("p (th fo j) -> p th fo j", th=3, fo=FK)
    wo_r = wo.rearrange("(co p) n -> p co n", p=P)       # [128, 4, 1024]
    yT_r = yT.rearrange("(oo p) s -> p oo s", p=P)       # [128, 8, 2048]

    phat_bufs = 1 if apply_mask else 2

    with tile.TileContext(nc) as tc:
        with tc.tile_pool(name="persist", bufs=1) as persist, \
             tc.tile_pool(name="work", bufs=1) as work, \
             tc.tile_pool(name="small", bufs=1) as small, \
             tc.tile_pool(name="phat", bufs=phat_bufs) as phatp, \
             tc.tile_pool(name="opool", bufs=2) as opool, \
             tc.tile_pool(name="ps", bufs=2, space="PSUM") as ps:

            QT = persist.tile([P, PAIRS, S], BF16)        # 16KB/part
            KTt = persist.tile([P, PAIRS, S], BF16)       # 16KB/part
            # packed V for the ones-column PV stationaries (24.1KB/part)
            Vp = persist.tile([P, NKT, PAIRS, VW], BF16)
            wo_t = persist.tile([P, HD // P, D], BF16)    # 8KB/part

            # ones + shared-zeros columns of the packed V
            nc.vector.memset(Vp[:, :, :, DK:2 * DK + 2], 0.0)
            nc.vector.memset(Vp[:, :, :, DK], 1.0)
            nc.vector.memset(Vp[:, :, :, 2 * DK + 1], 1.0)

            if qkv_bias:
                qkb_t = persist.tile([P, 2, PAIRS], F32)
                nc.sync.dma_start(
                    qkb_t, qkb.rearrange("t (pr p) -> p t pr", p=P))
                vb_bc = persist.tile([P, HD], F32)
                nc.sync.dma_start(vb_bc, vb[None, :].partition_broadcast(P))

            # x and weights stay resident as per-ko tiles (fine-grained DMA
            # deps, no pool-close barriers anywhere)
            x_big = persist.tile([P, NTC, FK, TC], BF16, name="x_big")
            w_big = persist.tile([P, 3, FK, HD], BF16, name="w_big")
            # one fully-contiguous DMA per x quarter / w third
            nc.sync.dma_start(w_big[:, 1, :, 0:P], wqkv_r[:, 1, :, 0:P])
            nc.sync.dma_start(x_big[:, 0], xT_r[:, 0])
            nc.sync.dma_start(w_big[:, 1, :, P:HD], wqkv_r[:, 1, :, P:HD])
            nc.sync.dma_start(w_big[:, 0], wqkv_r[:, 0])
            nc.sync.dma_start(x_big[:, 1], xT_r[:, 1])
            nc.sync.dma_start(w_big[:, 2], wqkv_r[:, 2])
            nc.sync.dma_start(x_big[:, 2], xT_r[:, 2])
            nc.sync.dma_start(x_big[:, 3], xT_r[:, 3])
            nc.sync.dma_start(wo_t, wo_r)

            def emit_qk_group(which, pair, tcix):
                tsl = slice(tcix * TC, (tcix + 1) * TC)
                psqk = ps.tile([P, 2 * QC], F32, tag="o", name="psqk")[:, :TC]
                msl = slice(pair * P, (pair + 1) * P)
                for ko in range(FK):
                    nc.tensor.matmul(
                        psqk, w_big[:, which, ko, msl],
                        x_big[:, tcix, ko], start=(ko == 0),
                        stop=(ko == FK - 1))
                dst = (QT if which == 0 else KTt)[:, pair, tsl]
                if qkv_bias:
                    nc.vector.tensor_scalar_add(
                        dst, psqk, qkb_t[:, which, pair, None])
                else:
                    nc.vector.tensor_copy(dst, psqk)

            def emit_v_group(kt):
                psv = ps.tile([P, 2 * QC], F32, tag="o", name="psv")[:, :HD]
                ql, off = kt // 4, (kt % 4) * P
                for ko in range(FK):
                    nc.tensor.matmul(
                        psv, x_big[:, ql, ko, off:off + P],
                        w_big[:, 2, ko], start=(ko == 0),
                        stop=(ko == FK - 1))
                vsrc = psv.rearrange("p (pr t c) -> p pr t c", pr=PAIRS, t=2)
                if qkv_bias:
                    vbr = vb_bc.rearrange(
                        "p (pr t c) -> p pr t c", pr=PAIRS, t=2)
                    nc.vector.tensor_add(
                        Vp[:, kt, :, 0:DK], vsrc[:, :, 0], vbr[:, :, 0])
                    nc.vector.tensor_add(
                        Vp[:, kt, :, 2 * DK + 2:VW], vsrc[:, :, 1],
                        vbr[:, :, 1])
                else:
                    nc.vector.tensor_copy(
                        Vp[:, kt, :, 0:DK], vsrc[:, :, 0])
                    nc.vector.tensor_copy(
                        Vp[:, kt, :, 2 * DK + 2:VW], vsrc[:, :, 1])

            o_tiles = {}

            def emit_proj_group(qc, oc):
                qsl = slice(qc * QC, (qc + 1) * QC)
                psy = ps.tile([P, 2 * QC], F32, tag="o", name="psy")[:, :QC]
                for c in range(HD // P):
                    nc.tensor.matmul(
                        psy, wo_t[:, c, oc * P:(oc + 1) * P],
                        o_tiles[qc][:, c, :],
                        start=(c == 0), stop=(c == HD // P - 1))
                yst = work.tile([P, QC], BF16, tag=f"y{oc % 2}")
                nc.vector.tensor_copy(yst, psy)
                nc.sync.dma_start(yT_r[:, oc, qsl], yst)

            def emit_tail(st, pod):
                # denominators: bank A partition 64, bank B partition 63
                rr = small.tile([DK + 1, 2 * QC], F32, tag="r")
                # rr row 0 is recip garbage; reuse it as the partition-0
                # staging slot for the two denominator reciprocals
                nc.vector.reciprocal_approx_fast(
                    rr[0:DK + 1, 0:QC], pod[0:DK + 1, 0:QC])
                nc.sync.dma_start(rr[0:1, 0:QC], rr[DK:DK + 1, 0:QC])
                nc.vector.reciprocal_approx_fast(
                    rr[0:DK, QC:2 * QC], pod[0:DK, QC:2 * QC])
                nc.sync.dma_start(rr[0:1, QC:2 * QC],
                                  rr[DK - 1:DK, QC:2 * QC])
                rbcA = small.tile([DK, QC], F32, tag="rbcA")
                rbcB = small.tile([P, QC], F32, tag="rbcB")
                nc.gpsimd.partition_broadcast(rbcA, rr[0:1, 0:QC])
                nc.gpsimd.partition_broadcast(rbcB, rr[0:1, QC:2 * QC])
                nc.vector.tensor_mul(
                    o_tiles[st["qc"]][0:DK, st["pair"], :],
                    pod[0:DK, 0:QC], rbcA)
                nc.vector.tensor_mul(
                    o_tiles[st["qc"]][DK:P, st["pair"], :],
                    pod[DK:P, QC:2 * QC], rbcB[DK:P])
                if dbg and (st["qc"], st["pair"]) == dbg_it:
                    dent = work.tile([P, QC], F32, tag="y0")
                    nc.vector.tensor_copy(dent[0:DK + 1, :],
                                          pod[0:DK + 1, 0:QC])
                    nc.sync.dma_start(dbg_den[0:1, :], dent[DK:DK + 1])
                    nc.vector.tensor_copy(dent[0:DK, :],
                                          pod[0:DK, QC:2 * QC])
                    nc.sync.dma_start(dbg_den[1:2, :], dent[DK - 1:DK])
                    nc.sync.dma_start(dbg_r[0:1, :], rr[0:1, 0:QC])
                    nc.sync.dma_start(dbg_r[1:2, :], rr[0:1, QC:2 * QC])
                    ot = work.tile([P, QC], F32, tag="y0")
                    nc.vector.tensor_copy(
                        ot, o_tiles[st["qc"]][:, st["pair"], :])
                    nc.sync.dma_start(dbg_o[:, :], ot)

            # ---- prologue: K^T (all chunks) + Q^T (chunk 0) -------------
            for tcix in range(NTC):
                for pair in range(PAIRS):
                    emit_qk_group(1, pair, tcix)
                if tcix == 0:
                    for pair in range(PAIRS):
                        emit_qk_group(0, pair, 0)

            # V and remaining Q drain into the attention kt-stream
            pending = []   # (ready_iter, late_only, fn)
            for kt in range(NKT):
                pending.append((0, False, (lambda kt=kt: emit_v_group(kt))))
            for tcix in range(1, NTC):
                for pair in range(PAIRS):
                    pending.append(
                        (0, False, (lambda pair=pair, tcix=tcix:
                                    emit_qk_group(0, pair, tcix))))
            it_idx = 0

            def drain(limit, allow_late=True):
                n = 0
                while (pending and n < limit and pending[0][0] <= it_idx
                       and (allow_late or not pending[0][1])):
                    _, _, fn = pending.pop(0)
                    fn()
                    n += 1

            def emit_pv(st, pod, kt, first, last):
                pv = st["phat"][kt]
                vk = Vp[:, kt, st["pair"]]
                nc.tensor.matmul(
                    pod[:, 0:QC], vk[:, 0:P], pv[:, 0:QC],
                    start=first, stop=last)
                nc.tensor.matmul(
                    pod[:, QC:2 * QC], vk[:, DK + 2:VW], pv[:, QC:2 * QC],
                    start=first, stop=last)

            # ---- main attention pipeline --------------------------------
            prev = None
            for qc in range(NQC):
                qsl = slice(qc * QC, (qc + 1) * QC)
                o_tiles[qc] = opool.tile(
                    [P, HD // P, QC], BF16, tag="o_sb", name="o_sb")
                if apply_mask:
                    mt = opool.tile([P, NKT, QC], F32, tag="mask")
                    nc.sync.dma_start(
                        mt,
                        maskT.rearrange("(ko p) q -> p ko q", p=P)[:, :, qsl])
                for pair in range(PAIRS):
                    phat = [phatp.tile([P, 2 * QC], BF16, tag=f"ph{k}",
                                       name=f"ph{k}") for k in range(NKT)]
                    pod = (ps.tile([P, 2 * QC], F32, tag="o", name="pod")
                           if prev is not None else None)
                    for kt2 in range(0, NKT, 2):
                        for kt in (kt2, kt2 + 1):
                            ksl = slice(kt * P, (kt + 1) * P)
                            pss = ps.tile([P, 2 * QC], F32, tag="scores",
                                          name=f"pss{kt & 1}")
                            nc.tensor.matmul(
                                pss[:, 0:QC], KTt[0:DK, pair, ksl],
                                QT[0:DK, pair, qsl], start=True, stop=True)
                            nc.tensor.matmul(
                                pss[:, QC:2 * QC], KTt[DK:P, pair, ksl],
                                QT[DK:P, pair, qsl], start=True, stop=True)
                            if apply_mask:
                                nc.vector.tensor_add(
                                    pss[:, 0:QC], pss[:, 0:QC], mt[:, kt])
                                nc.vector.tensor_add(
                                    pss[:, QC:2 * QC], pss[:, QC:2 * QC],
                                    mt[:, kt])
                            if kt in dve_kt:
                                nc.vector.tensor_scalar(
                                    phat[kt].bitcast(I16), pss, SCH_A, SCH_B,
                                    mybir.AluOpType.mult,
                                    mybir.AluOpType.add)
                            else:
                                nc.scalar.activation(
                                    phat[kt], pss,
                                    mybir.ActivationFunctionType.Exp)
                        if it_idx <= 1:
                            drain(1, allow_late=(prev is None or kt2 >= 5))
                        elif (kt2 // 2) % 2 == 0:
                            drain(1, allow_late=(kt2 >= 5))
                        if prev is not None:
                            emit_pv(prev, pod, kt2, kt2 == 0, False)
                            emit_pv(prev, pod, kt2 + 1, False,
                                    kt2 + 1 == NKT - 1)
                    if prev is not None:
                        emit_tail(prev, pod)
                        if prev["pair"] == PAIRS - 1:
                            pending.extend(
                                (it_idx + 1, True,
                                 (lambda pqc=prev["qc"], oc=oc:
                                  emit_proj_group(pqc, oc)))
                                for oc in range(D // P))
                    prev = {"qc": qc, "pair": pair, "phat": phat}
                    it_idx += 1

            # epilogue
            pod = ps.tile([P, 2 * QC], F32, tag="o", name="pod")
            it_idx += 100
            for kt in range(NKT):
                emit_pv(prev, pod, kt, kt == 0, kt == NKT - 1)
            emit_tail(prev, pod)
            while pending:
                _, _, fn = pending.pop(0)
                fn()
            for oc in range(D // P):
                emit_proj_group(prev["qc"], oc)

    nc.finalize()
    return nc


# --------------------------------------------------------------------------
# NTFF profiling shim (only used when kernel(..., _trace=True); provides
# antenv.axon_hooks so run_bass_kernel_spmd can capture profiles under axon).
def _install_ntff_shim():
    import contextlib, ctypes, sys, types
    try:
        import antenv.axon_hooks  # noqa: F401
        return
    except ImportError:
        pass
    so = "/opt/axon/libaxon_pjrt.so"
    try:
        lib = ctypes.CDLL(so)
    except OSError:
        return
    if not hasattr(lib, "axon_start_nrt_profile"):
        return
    lib.axon_start_nrt_profile.argtypes = [
        ctypes.POINTER(ctypes.c_int64), ctypes.c_size_t]
    lib.axon_start_nrt_profile.restype = ctypes.c_int64
    lib.axon_stop_nrt_profile.argtypes = [ctypes.c_char_p]
    lib.axon_stop_nrt_profile.restype = ctypes.c_int64

    @contextlib.contextmanager
    def _hook(output_dir, device_ids):
        import jax
        jax.devices()
        if device_ids:
            ids = (ctypes.c_int64 * len(device_ids))(*device_ids)
            rc = lib.axon_start_nrt_profile(ids, len(device_ids))
        else:
            rc = lib.axon_start_nrt_profile(None, 0)
        if rc != 0:
            raise RuntimeError(f"axon_start_nrt_profile rc={rc}")
        try:
            yield
        finally:
            n = lib.axon_stop_nrt_profile(str(output_dir).encode())
            print(f"ntff: {n} profile file(s) in {output_dir}", file=sys.stderr)

    import antenv
    mod = types.ModuleType("antenv.axon_hooks")
    mod.get_axon_ntff_profile_hook = lambda: _hook
    mod.set_axon_ntff_profile_hook = lambda h: None
    sys.modules["antenv.axon_hooks"] = mod
    antenv.axon_hooks = mod


def kernel(x, mask, Wq, bq, Wk, bk, Wv, bv, Wo, bo, _trace=False):
    global LAST_EXEC_NS
    x = np.ascontiguousarray(np.asarray(x, dtype=np.float32))
    mask = np.asarray(mask)
    Wq = np.asarray(Wq, dtype=np.float32)
    Wk = np.asarray(Wk, dtype=np.float32)
    Wv = np.asarray(Wv, dtype=np.float32)
    Wo = np.asarray(Wo, dtype=np.float32)
    bq = np.asarray(bq, dtype=np.float32)
    bk = np.asarray(bk, dtype=np.float32)
    bv = np.asarray(bv, dtype=np.float32)
    bo = np.asarray(bo, dtype=np.float32)

    scale = np.float32(1.0 / math.sqrt(DK))
    apply_mask = not bool((mask != 0).all())
    qkv_bias = bool(bq.any() or bk.any() or bv.any())

    import os
    dve_kt = tuple(
        int(t) for t in os.environ.get("DVE_KT", "5,9,12,15").split(",") if t)
    dbg = bool(os.environ.get("KDBG"))
    dbg_it = tuple(int(t) for t in os.environ.get("DBGIT", "0,0").split(","))
    nc = _build(apply_mask, qkv_bias, dve_kt=dve_kt, dbg=dbg, dbg_it=dbg_it)

    if apply_mask:
        mbias = np.where(mask == 0, np.float32(-1e9), np.float32(0.0))
        # maskT[b][k, q] = mbias[b][q, k]
        maskT = np.ascontiguousarray(np.transpose(mbias, (0, 2, 1)))

    in_maps = []
    for b in range(B):
        # pack x as [p, quarter, ko, t] so each quarter is one contiguous DMA
        xT_np = np.ascontiguousarray(
            x[b].reshape(NTC, TC, FK, P).transpose(3, 0, 2, 1)
            .reshape(P, -1)).astype(ml_dtypes.bfloat16)
        for g in range(HG):
            rows = slice(g * HD, (g + 1) * HD)
            wc = np.concatenate(
                [Wq[rows].T * scale, Wk[rows].T, Wv[rows].T],
                axis=1)   # [D, 3*HD], thirds Q|K|V
            wqkv_np = np.ascontiguousarray(
                wc.reshape(FK, P, 3, HD).transpose(1, 2, 0, 3)
                .reshape(P, -1)).astype(ml_dtypes.bfloat16)
            wo_np = np.ascontiguousarray(
                Wo[:, rows].T).astype(ml_dtypes.bfloat16)
            m = {"xT": xT_np, "wqkv": wqkv_np, "wo": wo_np}
            if apply_mask:
                m["maskT"] = maskT[b]
            if qkv_bias:
                m["qkb"] = np.ascontiguousarray(
                    np.stack([bq[rows] * scale, bk[rows]]))
                m["vb"] = np.ascontiguousarray(bv[rows])
            in_maps.append(m)

    if _trace:
        _install_ntff_shim()
    r = run_bass_kernel_spmd(nc, in_maps, list(range(NCORES)), trace=_trace)
    LAST_EXEC_NS = r.exec_time_ns
    if dbg:
        global DBG_OUT
        DBG_OUT = r.results

    y = np.empty((B, S, D), dtype=np.float32)
    for b in range(B):
        yT = (r.results[2 * b]["yT"].astype(np.float32)
              + r.results[2 * b + 1]["yT"].astype(np.float32))
        y[b] = yT.T + bo[None, :]
    return y


# revision 39
# speedup vs baseline: 1.0165x; 1.0103x over previous
"""Multi-head attention (B=4, S=2048, D=1024, H=16) on 8 trn2 NeuronCores.

Sharding: core c = (batch b, head-group g) with b in 0..3, g in 0..1.
Each core computes 8 heads of one batch; the two cores of a batch produce
partial output projections that the host sums.

All device tensors are kept in "transposed" layouts (feature dim on SBUF
partitions) so no on-device transposes are needed:
  Q^T/K^T [d, s], V [s, d], scores^T [k, q], o^T [d, q], y^T [out, q].

The two heads of a pair share each PE pass: QK^T is row-tiled over the two
K=64 halves of the partition dim (concurrent matmuls), and P·V uses two
128-column stationaries built from an overlapped V layout
  [V_A | 1 | 0...0 | 1 | V_B]   (193 columns per (kt, pair))
so both P·V matmuls keep Fast Weight Load (128-column weights) and carry
the softmax-denominator ones-column in fp32 PSUM for free:
  bank A out: o_A on partitions 0:64,  denom_A on partition 64
  bank B out: denom_B on partition 63, o_B on partitions 64:128.
Exp runs mostly on the scalar engine (ACT); a few tiles per iteration can
be offloaded to the vector engine via a Schraudolph-style bf16 bit-trick
exp to balance engine load.
"""
import math

import numpy as np
import ml_dtypes

import concourse.bass as bass
import concourse.mybir as mybir
import concourse.tile as tile
from concourse import bacc
from concourse.bass_utils import run_bass_kernel_spmd

B, S, D, H = 4, 2048, 1024, 16
DK = D // H              # 64
NCORES = 8
HG = 2                   # head groups (tensor-parallel axis)
HPG = H // HG            # 8 heads per core
HD = HPG * DK            # 512 head-dim features per core
PAIRS = HPG // 2         # 4 head pairs (2 heads packed per PE pass)
P = 128
VW = 194                 # packed V: [V_A |1| zeros |1| V_B] (B at 4B-aligned col 66)
QC = 512                 # q-chunk (matmul moving free dim)
NQC = S // QC            # 4
NKT = S // P             # 16 k-tiles
FK = D // P              # 8 feature c-tiles for projections
TC = 512                 # token chunk for QKV phase
NTC = S // TC            # 4

F32 = mybir.dt.float32
BF16 = mybir.dt.bfloat16
I16 = mybir.dt.int16

# Schraudolph bf16 exp: bits(exp(x)) ~= int16(x * 128*log2(e) + (127*128 - C))
SCH_A = 128.0 * 1.4426950408889634
SCH_B = 127.0 * 128.0 - 4.74

LAST_EXEC_NS = None


def _build(apply_mask: bool, qkv_bias: bool, dve_kt=(5, 9, 12, 15),
           dbg=False,
           dbg_it=(0, 0)):
    nc = bacc.Bacc("TRN2", debug=False, num_devices=NCORES)
    xT = nc.declare_dram_parameter("xT", [P, NTC * FK * TC], BF16,
                                   isOutput=False)
    wqkv = nc.declare_dram_parameter("wqkv", [P, 3 * FK * HD], BF16,
                                    isOutput=False)
    wo = nc.declare_dram_parameter("wo", [HD, D], BF16, isOutput=False)
    yT = nc.declare_dram_parameter("yT", [D, S], BF16, isOutput=True)
    if dbg:
        dbg_den = nc.declare_dram_parameter("dbg_den", [2, QC], F32,
                                            isOutput=True)
        dbg_r = nc.declare_dram_parameter("dbg_r", [2, QC], F32,
                                          isOutput=True)
        dbg_o = nc.declare_dram_parameter("dbg_o", [P, QC], F32,
                                          isOutput=True)
    if apply_mask:
        maskT = nc.declare_dram_parameter("maskT", [S, S], F32, isOutput=False)
        dve_kt = ()          # keep the masked path simple: all exp on ACT
    if qkv_bias:
        qkb = nc.declare_dram_parameter("qkb", [2, HD], F32, isOutput=False)
        vb = nc.declare_dram_parameter("vb", [HD], F32, isOutput=False)
    dve_kt = set(dve_kt)

    xT_r = xT.rearrange("p (q fo t) -> p q fo t", q=NTC, fo=FK)
    wqkv_r = wqkv.rearrange("p (th fo j) -> p th fo j", th=3, fo=FK)
    wo_r = wo.rearrange("(co p) n -> p co n", p=P)       # [128, 4, 1024]
    yT_r = yT.rearrange("(oo p) s -> p oo s", p=P)       # [128, 8, 2048]

    phat_bufs = 1 if apply_mask else 2

    with tile.TileContext(nc) as tc:
        with tc.tile_pool(name="persist", bufs=1) as persist, \
             tc.tile_pool(name="work", bufs=1) as work, \
             tc.tile_pool(name="small", bufs=1) as small, \
             tc.tile_pool(name="phat", bufs=phat_bufs) as phatp, \
             tc.tile_pool(name="opool", bufs=2) as opool, \
             tc.tile_pool(name="ps", bufs=2, space="PSUM") as ps:

            QT = persist.tile([P, PAIRS, S], BF16)        # 16KB/part
            KTt = persist.tile([P, PAIRS, S], BF16)       # 16KB/part
            # packed V for the ones-column PV stationaries (24.1KB/part)
            Vp = persist.tile([P, NKT, PAIRS, VW], BF16)
            wo_t = persist.tile([P, HD // P, D], BF16)    # 8KB/part

            # ones + shared-zeros columns of the packed V
            nc.vector.memset(Vp[:, :, :, DK:2 * DK + 2], 0.0)
            nc.vector.memset(Vp[:, :, :, DK], 1.0)
            nc.vector.memset(Vp[:, :, :, 2 * DK + 1], 1.0)

            if qkv_bias:
                qkb_t = persist.tile([P, 2, PAIRS], F32)
                nc.sync.dma_start(
                    qkb_t, qkb.rearrange("t (pr p) -> p t pr", p=P))
                vb_bc = persist.tile([P, HD], F32)
                nc.sync.dma_start(vb_bc, vb[None, :].partition_broadcast(P))

            # x and weights stay resident as per-ko tiles (fine-grained DMA
            # deps, no pool-close barriers anywhere)
            x_big = persist.tile([P, NTC, FK, TC], BF16, name="x_big")
            w_big = persist.tile([P, 3, FK, HD], BF16, name="w_big")
            # one fully-contiguous DMA per x quarter / w third
            nc.sync.dma_start(w_big[:, 1, :, 0:P], wqkv_r[:, 1, :, 0:P])
            nc.sync.dma_start(x_big[:, 0], xT_r[:, 0])
            nc.sync.dma_start(w_big[:, 1, :, P:HD], wqkv_r[:, 1, :, P:HD])
            nc.sync.dma_start(w_big[:, 0], wqkv_r[:, 0])
            nc.sync.dma_start(x_big[:, 1], xT_r[:, 1])
            nc.sync.dma_start(w_big[:, 2], wqkv_r[:, 2])
            nc.sync.dma_start(x_big[:, 2], xT_r[:, 2])
            nc.sync.dma_start(x_big[:, 3], xT_r[:, 3])
            nc.sync.dma_start(wo_t, wo_r)

            def emit_qk_group(which, pair, tcix):
                tsl = slice(tcix * TC, (tcix + 1) * TC)
                psqk = ps.tile([P, 2 * QC], F32, tag="o", name="psqk")[:, :TC]
                msl = slice(pair * P, (pair + 1) * P)
                for ko in range(FK):
                    nc.tensor.matmul(
                        psqk, w_big[:, which, ko, msl],
                        x_big[:, tcix, ko], start=(ko == 0),
                        stop=(ko == FK - 1))
                dst = (QT if which == 0 else KTt)[:, pair, tsl]
                if qkv_bias:
                    nc.vector.tensor_scalar_add(
                        dst, psqk, qkb_t[:, which, pair, None])
                else:
                    nc.vector.tensor_copy(dst, psqk)

            def emit_v_group(kt):
                psv = ps.tile([P, 2 * QC], F32, tag="o", name="psv")[:, :HD]
                ql, off = kt // 4, (kt % 4) * P
                for ko in range(FK):
                    nc.tensor.matmul(
                        psv, x_big[:, ql, ko, off:off + P],
                        w_big[:, 2, ko], start=(ko == 0),
                        stop=(ko == FK - 1))
                vsrc = psv.rearrange("p (pr t c) -> p pr t c", pr=PAIRS, t=2)
                if qkv_bias:
                    vbr = vb_bc.rearrange(
                        "p (pr t c) -> p pr t c", pr=PAIRS, t=2)
                    nc.vector.tensor_add(
                        Vp[:, kt, :, 0:DK], vsrc[:, :, 0], vbr[:, :, 0])
                    nc.vector.tensor_add(
                        Vp[:, kt, :, 2 * DK + 2:VW], vsrc[:, :, 1],
                        vbr[:, :, 1])
                else:
                    nc.vector.tensor_copy(
                        Vp[:, kt, :, 0:DK], vsrc[:, :, 0])
                    nc.vector.tensor_copy(
                        Vp[:, kt, :, 2 * DK + 2:VW], vsrc[:, :, 1])

            o_tiles = {}

            def emit_proj_group(qc, oc):
                qsl = slice(qc * QC, (qc + 1) * QC)
                psy = ps.tile([P, 2 * QC], F32, tag="o", name="psy")[:, :QC]
                for c in range(HD // P):
                    nc.tensor.matmul(
                        psy, wo_t[:, c, oc * P:(oc + 1) * P],
                        o_tiles[qc][:, c, :],
                        start=(c == 0), stop=(c == HD // P - 1))
                yst = work.tile([P, QC], BF16, tag=f"y{oc % 2}")
                nc.vector.tensor_copy(yst, psy)
                nc.sync.dma_start(yT_r[:, oc, qsl], yst)

            def emit_tail(st, pod):
                # denominators: bank A partition 64, bank B partition 63
                rr = small.tile([DK + 1, 2 * QC], F32, tag="r")
                # rr row 0 is recip garbage; reuse it as the partition-0
                # staging slot for the two denominator reciprocals
                nc.vector.reciprocal_approx_fast(
                    rr[0:DK + 1, 0:QC], pod[0:DK + 1, 0:QC])
                nc.sync.dma_start(rr[0:1, 0:QC], rr[DK:DK + 1, 0:QC])
                nc.vector.reciprocal_approx_fast(
                    rr[0:DK, QC:2 * QC], pod[0:DK, QC:2 * QC])
                nc.sync.dma_start(rr[0:1, QC:2 * QC],
                                  rr[DK - 1:DK, QC:2 * QC])
                rbcA = small.tile([DK, QC], F32, tag="rbcA")
                rbcB = small.tile([P, QC], F32, tag="rbcB")
                nc.gpsimd.partition_broadcast(rbcA, rr[0:1, 0:QC])
                nc.gpsimd.partition_broadcast(rbcB, rr[0:1, QC:2 * QC])
                nc.vector.tensor_mul(
                    o_tiles[st["qc"]][0:DK, st["pair"], :],
                    pod[0:DK, 0:QC], rbcA)
                nc.vector.tensor_mul(
                    o_tiles[st["qc"]][DK:P, st["pair"], :],
                    pod[DK:P, QC:2 * QC], rbcB[DK:P])
                if dbg and (st["qc"], st["pair"]) == dbg_it:
                    dent = work.tile([P, QC], F32, tag="y0")
                    nc.vector.tensor_copy(dent[0:DK + 1, :],
                                          pod[0:DK + 1, 0:QC])
                    nc.sync.dma_start(dbg_den[0:1, :], dent[DK:DK + 1])
                    nc.vector.tensor_copy(dent[0:DK, :],
                                          pod[0:DK, QC:2 * QC])
                    nc.sync.dma_start(dbg_den[1:2, :], dent[DK - 1:DK])
                    nc.sync.dma_start(dbg_r[0:1, :], rr[0:1, 0:QC])
                    nc.sync.dma_start(dbg_r[1:2, :], rr[0:1, QC:2 * QC])
                    ot = work.tile([P, QC], F32, tag="y0")
                    nc.vector.tensor_copy(
                        ot, o_tiles[st["qc"]][:, st["pair"], :])
                    nc.sync.dma_start(dbg_o[:, :], ot)

            # ---- prologue: K^T (all chunks) + Q^T (chunk 0) -------------
            for tcix in range(NTC):
                for pair in range(PAIRS):
                    emit_qk_group(1, pair, tcix)
                if tcix == 0:
                    for pair in range(PAIRS):
                        emit_qk_group(0, pair, 0)

            # V and remaining Q drain into the attention kt-stream
            pending = []   # (ready_iter, late_only, fn)
            for kt in range(NKT):
                pending.append((0, False, (lambda kt=kt: emit_v_group(kt))))
            for tcix in range(1, NTC):
                for pair in range(PAIRS):
                    pending.append(
                        (0, False, (lambda pair=pair, tcix=tcix:
                                    emit_qk_group(0, pair, tcix))))
            it_idx = 0

            def drain(limit, allow_late=True):
                n = 0
                while (pending and n < limit and pending[0][0] <= it_idx
                       and (allow_late or not pending[0][1])):
                    _, _, fn = pending.pop(0)
                    fn()
                    n += 1

            def emit_pv(st, pod, kt, first, last):
                pv = st["phat"][kt]
                vk = Vp[:, kt, st["pair"]]
                nc.tensor.matmul(
                    pod[:, 0:QC], vk[:, 0:P], pv[:, 0:QC],
                    start=first, stop=last)
                nc.tensor.matmul(
                    pod[:, QC:2 * QC], vk[:, DK + 2:VW], pv[:, QC:2 * QC],
                    start=first, stop=last)

            # ---- main attention pipeline --------------------------------
            prev = None
            for qc in range(NQC):
                qsl = slice(qc * QC, (qc + 1) * QC)
                o_tiles[qc] = opool.tile(
                    [P, HD // P, QC], BF16, tag="o_sb", name="o_sb")
                if apply_mask:
                    mt = opool.tile([P, NKT, QC], F32, tag="mask")
                    nc.sync.dma_start(
                        mt,
                        maskT.rearrange("(ko p) q -> p ko q", p=P)[:, :, qsl])
                for pair in range(PAIRS):
                    phat = [phatp.tile([P, 2 * QC], BF16, tag=f"ph{k}",
                                       name=f"ph{k}") for k in range(NKT)]
                    pod = (ps.tile([P, 2 * QC], F32, tag="o", name="pod")
                           if prev is not None else None)
                    for kt2 in range(0, NKT, 2):
                        for kt in (kt2, kt2 + 1):
                            ksl = slice(kt * P, (kt + 1) * P)
                            pss = ps.tile([P, 2 * QC], F32, tag="scores",
                                          name=f"pss{kt & 1}")
                            nc.tensor.matmul(
                                pss[:, 0:QC], KTt[0:DK, pair, ksl],
                                QT[0:DK, pair, qsl], start=True, stop=True)
                            nc.tensor.matmul(
                                pss[:, QC:2 * QC], KTt[DK:P, pair, ksl],
                                QT[DK:P, pair, qsl], start=True, stop=True)
                            if apply_mask:
                                nc.vector.tensor_add(
                                    pss[:, 0:QC], pss[:, 0:QC], mt[:, kt])
                                nc.vector.tensor_add(
                                    pss[:, QC:2 * QC], pss[:, QC:2 * QC],
                                    mt[:, kt])
                            if kt in dve_kt:
                                nc.vector.tensor_scalar(
                                    phat[kt].bitcast(I16), pss, SCH_A, SCH_B,
                                    mybir.AluOpType.mult,
                                    mybir.AluOpType.add)
                            else:
                                nc.scalar.activation(
                                    phat[kt], pss,
                                    mybir.ActivationFunctionType.Exp)
                        if it_idx <= 1:
                            drain(1, allow_late=(prev is None or kt2 >= 5))
                        elif (kt2 // 2) % 2 == 1:
                            drain(1, allow_late=(kt2 >= 7))
                        if prev is not None:
                            emit_pv(prev, pod, kt2, kt2 == 0, False)
                            emit_pv(prev, pod, kt2 + 1, False,
                                    kt2 + 1 == NKT - 1)
                    if prev is not None:
                        emit_tail(prev, pod)
                        if prev["pair"] == PAIRS - 1:
                            pending.extend(
                                (it_idx + 1, True,
                                 (lambda pqc=prev["qc"], oc=oc:
                                  emit_proj_group(pqc, oc)))
                                for oc in range(D // P))
                    prev = {"qc": qc, "pair": pair, "phat": phat}
                    it_idx += 1

            # epilogue
            pod = ps.tile([P, 2 * QC], F32, tag="o", name="pod")
            it_idx += 100
            for kt in range(NKT):
                emit_pv(prev, pod, kt, kt == 0, kt == NKT - 1)
            emit_tail(prev, pod)
            while pending:
                _, _, fn = pending.pop(0)
                fn()
            for oc in range(D // P):
                emit_proj_group(prev["qc"], oc)

    nc.finalize()
    return nc


# --------------------------------------------------------------------------
# NTFF profiling shim (only used when kernel(..., _trace=True); provides
# antenv.axon_hooks so run_bass_kernel_spmd can capture profiles under axon).
def _install_ntff_shim():
    import contextlib, ctypes, sys, types
    try:
        import antenv.axon_hooks  # noqa: F401
        return
    except ImportError:
        pass
    so = "/opt/axon/libaxon_pjrt.so"
    try:
        lib = ctypes.CDLL(so)
    except OSError:
        return
    if not hasattr(lib, "axon_start_nrt_profile"):
        return
    lib.axon_start_nrt_profile.argtypes = [
        ctypes.POINTER(ctypes.c_int64), ctypes.c_size_t]
    lib.axon_start_nrt_profile.restype = ctypes.c_int64
    lib.axon_stop_nrt_profile.argtypes = [ctypes.c_char_p]
    lib.axon_stop_nrt_profile.restype = ctypes.c_int64

    @contextlib.contextmanager
    def _hook(output_dir, device_ids):
        import jax
        jax.devices()
        if device_ids:
            ids = (ctypes.c_int64 * len(device_ids))(*device_ids)
            rc = lib.axon_start_nrt_profile(ids, len(device_ids))
        else:
            rc = lib.axon_start_nrt_profile(None, 0)
        if rc != 0:
            raise RuntimeError(f"axon_start_nrt_profile rc={rc}")
        try:
            yield
        finally:
            n = lib.axon_stop_nrt_profile(str(output_dir).encode())
            print(f"ntff: {n} profile file(s) in {output_dir}", file=sys.stderr)

    import antenv
    mod = types.ModuleType("antenv.axon_hooks")
    mod.get_axon_ntff_profile_hook = lambda: _hook
    mod.set_axon_ntff_profile_hook = lambda h: None
    sys.modules["antenv.axon_hooks"] = mod
    antenv.axon_hooks = mod


def kernel(x, mask, Wq, bq, Wk, bk, Wv, bv, Wo, bo, _trace=False):
    global LAST_EXEC_NS
    x = np.ascontiguousarray(np.asarray(x, dtype=np.float32))
    mask = np.asarray(mask)
    Wq = np.asarray(Wq, dtype=np.float32)
    Wk = np.asarray(Wk, dtype=np.float32)
    Wv = np.asarray(Wv, dtype=np.float32)
    Wo = np.asarray(Wo, dtype=np.float32)
    bq = np.asarray(bq, dtype=np.float32)
    bk = np.asarray(bk, dtype=np.float32)
    bv = np.asarray(bv, dtype=np.float32)
    bo = np.asarray(bo, dtype=np.float32)

    scale = np.float32(1.0 / math.sqrt(DK))
    apply_mask = not bool((mask != 0).all())
    qkv_bias = bool(bq.any() or bk.any() or bv.any())

    import os
    dve_kt = tuple(
        int(t) for t in os.environ.get("DVE_KT", "5,9,12,15").split(",") if t)
    dbg = bool(os.environ.get("KDBG"))
    dbg_it = tuple(int(t) for t in os.environ.get("DBGIT", "0,0").split(","))
    nc = _build(apply_mask, qkv_bias, dve_kt=dve_kt, dbg=dbg, dbg_it=dbg_it)

    if apply_mask:
        mbias = np.where(mask == 0, np.float32(-1e9), np.float32(0.0))
        # maskT[b][k, q] = mbias[b][q, k]
        maskT = np.ascontiguousarray(np.transpose(mbias, (0, 2, 1)))

    in_maps = []
    for b in range(B):
        # pack x as [p, quarter, ko, t] so each quarter is one contiguous DMA
        xT_np = np.ascontiguousarray(
            x[b].reshape(NTC, TC, FK, P).transpose(3, 0, 2, 1)
            .reshape(P, -1)).astype(ml_dtypes.bfloat16)
        for g in range(HG):
            rows = slice(g * HD, (g + 1) * HD)
            wc = np.concatenate(
                [Wq[rows].T * scale, Wk[rows].T, Wv[rows].T],
                axis=1)   # [D, 3*HD], thirds Q|K|V
            wqkv_np = np.ascontiguousarray(
                wc.reshape(FK, P, 3, HD).transpose(1, 2, 0, 3)
                .reshape(P, -1)).astype(ml_dtypes.bfloat16)
            wo_np = np.ascontiguousarray(
                Wo[:, rows].T).astype(ml_dtypes.bfloat16)
            m = {"xT": xT_np, "wqkv": wqkv_np, "wo": wo_np}
            if apply_mask:
                m["maskT"] = maskT[b]
            if qkv_bias:
                m["qkb"] = np.ascontiguousarray(
                    np.stack([bq[rows] * scale, bk[rows]]))
                m["vb"] = np.ascontiguousarray(bv[rows])
            in_maps.append(m)

    if _trace:
        _install_ntff_shim()
    r = run_bass_kernel_spmd(nc, in_maps, list(range(NCORES)), trace=_trace)
    LAST_EXEC_NS = r.exec_time_ns
    if dbg:
        global DBG_OUT
        DBG_OUT = r.results

    y = np.empty((B, S, D), dtype=np.float32)
    for b in range(B):
        yT = (r.results[2 * b]["yT"].astype(np.float32)
              + r.results[2 * b + 1]["yT"].astype(np.float32))
        y[b] = yT.T + bo[None, :]
    return y
